# revision 1
# baseline (speedup 1.0000x reference)
"""ABCNN1 attention kernel for 8 Trainium2 NeuronCores.

Reference computation (per batch b of 64, with L=512, D=1024):
    S  = X1 @ X2^T                          (512 x 512)
    A  = S / (|X1_rows| outer |X2_rows|)    cosine match-score
    a1 = A @ W1            a2 = A^T @ W2    (512 x 1024 each)
    attn1 = concat([x1, a1], axis=1)        attn2 = concat([x2, a2], axis=1)

Device strategy (data-parallel, 8 batches per core, no collectives):
  - Host pre-transposes x1/x2 to [b, D, L] (bf16) so the contraction dim d
    lands on SBUF partitions with fully contiguous DMA, at half the HBM
    traffic of f32.
  - Only S is computed by matmul (32 MMs/batch); S^T comes from 16 cheap
    PE transpose-mode ops on the bf16 S tiles (vs 32 more matmuls).
  - Norms: squares (ACT) + bf16 pair-add tree (DVE) + one ones^T @ xsq_acc
    matmul per input -> nsq row [1, 512]; sqrt doubles as the PSUM->SBUF
    copy; a 4KB DRAM bounce scatters the rows into per-partition [128, 8]
    columns where the reciprocal is cheap.
  - All four normalization diagonals are folded into per-partition scalar
    multiplies (per-batch W1n = diag(r2) W1 and W2n = diag(r1) W2 as
    stage-2 rhs; r1/r2 row scales on the stage-2 output copies). The
    S/S^T PSUM->SBUF copies are plain, so the matmul pipeline never waits
    on the norm latency chain. No cross-partition broadcasts anywhere.
  - Matmuls run in bf16 (full TensorE rate, FWL weight loads); PSUM
    accumulation is f32. Whole-pipeline rel err ~2.3e-3, dominated by the
    bf16 input/output quantization.
  - Per-batch phases are software-pipelined one batch deep: batch b's
    input DMAs + squares land while b-1 computes; its norm tail is
    emitted between b-1's S-phase and stage 2 so no engine head-of-line
    blocks on it.
  - The concat halves of the outputs are just the inputs; they are
    assembled on host - the device computes and writes only a1/a2 (bf16).
"""

import numpy as np

B, L, D = 64, 512, 1024
N_CORES = 8
BB = B // N_CORES        # batches per core
KT = D // 128            # contraction tiles (d)
LT = L // 128            # row tiles (l or m)
NT = D // 512            # output free-dim chunks

_CACHE = {}


def _build(bb):
    import concourse.mybir as mybir
    import concourse.tile as tile
    from concourse import bacc
    from concourse import masks

    F32 = mybir.dt.float32
    F32R = mybir.dt.float32r
    BF16 = mybir.dt.bfloat16

    nc = bacc.Bacc("TRN2", target_bir_lowering=False, debug=False,
                   num_devices=N_CORES)
    x1t = nc.declare_dram_parameter("x1t", [bb, D, L], BF16, isOutput=False)
    x2t = nc.declare_dram_parameter("x2t", [bb, D, L], BF16, isOutput=False)
    w1 = nc.declare_dram_parameter("w1", [L, D], F32, isOutput=False)
    w2 = nc.declare_dram_parameter("w2", [L, D], F32, isOutput=False)
    out1 = nc.declare_dram_parameter("out1", [bb, L, D], BF16, isOutput=True)
    out2 = nc.declare_dram_parameter("out2", [bb, L, D], BF16, isOutput=True)

    with tile.TileContext(nc) as tc:
        with (
            tc.tile_pool(name="const", bufs=1) as constp,
            tc.tile_pool(name="xin", bufs=2) as xin,
            tc.tile_pool(name="sq", bufs=2) as sqp,
            tc.tile_pool(name="alhs", bufs=4) as alhsp,
            tc.tile_pool(name="aout", bufs=8) as aoutp,
            tc.tile_pool(name="small", bufs=3) as smallp,
            tc.tile_pool(name="dram", bufs=3, space="DRAM") as dramp,
            tc.tile_pool(name="ps_s", bufs=2, space="PSUM") as ps_s,
            tc.tile_pool(name="ps_t", bufs=1, space="PSUM") as ps_t,
            tc.tile_pool(name="ps_nsq", bufs=1, space="PSUM") as ps_nsq,
            tc.tile_pool(name="ps_a", bufs=3, space="PSUM") as ps_a,
        ):
            Copy = mybir.ActivationFunctionType.Copy

            # ---- persistent tiles -------------------------------------
            w1_sb = constp.tile([128, LT, D], BF16, tag="w1")
            w2_sb = constp.tile([128, LT, D], BF16, tag="w2")

            def emit_w_loads():
                for j in range(LT):
                    nc.gpsimd.dma_start(w1_sb[:, j, :], w1[128 * j:128 * (j + 1), :])
                    nc.gpsimd.dma_start(w2_sb[:, j, :], w2[128 * j:128 * (j + 1), :])

            ones_sb = constp.tile([128, 1], BF16, tag="ones")
            nc.gpsimd.memset(ones_sb[:], 1.0)
            ident_sb = constp.tile([128, 128], BF16, tag="ident")
            masks.make_identity(nc, ident_sb[:])

            def emit_load_sq(b):
                """Input DMAs (per k-slice) + squares/add-tree -> xsq accums."""
                x1r = xin.tile([128, KT, L], BF16, tag="x1", name="x1r")
                x2r = xin.tile([128, KT, L], BF16, tag="x2", name="x2r")
                for k in range(KT):
                    nc.sync.dma_start(x1r[:, k, :], x1t[b, 128 * k:128 * (k + 1), :])
                    nc.sync.dma_start(x2r[:, k, :], x2t[b, 128 * k:128 * (k + 1), :])

                accs = []
                for xi, x_r in enumerate((x1r, x2r)):
                    xsq = [sqp.tile([128, L], BF16, tag=f"xsq{xi}_{k}",
                                    name=f"xsq{xi}_{k}") for k in range(KT)]
                    for k in range(KT):
                        nc.scalar.square(xsq[k][:], x_r[:, k, :])
                    accs.append(xsq)
                return x1r, x2r, accs

            def emit_norm_tail(accs):
                """ones^T @ xsq_acc matmuls -> sqrt -> DRAM-bounce scatter ->
                reciprocal; returns the per-partition [128, 8] scale tile."""
                nsq1 = ps_nsq.tile([1, L], F32, tag="nsq1", name="nsq1")
                nsq2 = ps_nsq.tile([1, L], F32, tag="nsq2", name="nsq2")
                for xsq in accs:
                    step = 1
                    while step < KT:
                        for k in range(0, KT, 2 * step):
                            nc.vector.tensor_add(xsq[k][:], xsq[k][:], xsq[k + step][:])
                        step *= 2
                nc.tensor.matmul(nsq1[:], ones_sb[:], accs[0][0][:], start=True, stop=True)
                nc.tensor.matmul(nsq2[:], ones_sb[:], accs[1][0][:], start=True, stop=True)
                srow = smallp.tile([1, 2 * L], F32, tag="srow", name="srow")
                nc.scalar.sqrt(srow[:, 0:L], nsq1[:])
                nc.scalar.sqrt(srow[:, L:2 * L], nsq2[:])
                r_dram = dramp.tile([1, 2 * L], F32, tag="rd", name="r_dram")
                nc.scalar.dma_start(r_dram[:], srow[:])
                rst_sb = smallp.tile([128, 2 * LT], F32, tag="rst", name="rst_sb")
                nc.scalar.dma_start(rst_sb[:], r_dram.rearrange("o (c p) -> (o p) c", p=128))
                r_sb = smallp.tile([128, 2 * LT], F32, tag="rsb", name="r_sb")
                nc.vector.reciprocal(r_sb[:], rst_sb[:])
                # per-batch scaled weights: W1n = diag(r2) W1, W2n = diag(r1) W2
                w1n_sb = alhsp.tile([128, LT, D], BF16, tag="w1n", name="w1n_sb", bufs=3)
                w2n_sb = alhsp.tile([128, LT, D], BF16, tag="w2n", name="w2n_sb", bufs=3)
                for j in range(LT):
                    nc.vector.tensor_scalar_mul(w1n_sb[:, j, :], w1_sb[:, j, :],
                                                r_sb[:, LT + j:LT + j + 1])
                    nc.vector.tensor_scalar_mul(w2n_sb[:, j, :], w2_sb[:, j, :],
                                                r_sb[:, j:j + 1])
                return r_sb, w1n_sb, w2n_sb

            def emit_s_matmuls(x1r, x2r):
                """a2lhs = S (plain PSUM->SBUF copies, no norm dependency);
                a1lhs = S^T via PE transposes. All normalization diagonals
                are folded into W1n/W2n (rhs) and the output copies:
                  a1 = (S^T)^T (r2*W1) then r1-scaled rows = D1' S D2' W1
                  a2 = (S)^T  (r1*W2) then r2-scaled rows = D2' S^T D1' W2
                """
                a2lhs = alhsp.tile([128, LT, L], BF16, tag="a2lhs", name="a2lhs", bufs=6)
                a1lhs = alhsp.tile([128, LT, L], BF16, tag="a1lhs", name="a1lhs", bufs=6)
                for i in range(LT):
                    s_ps = ps_s.tile([128, L], F32, tag="s", name="s_ps")
                    for k in range(KT):
                        nc.tensor.matmul(s_ps[:], x1r[:, k, 128 * i:128 * (i + 1)],
                                         x2r[:, k, :], start=(k == 0), stop=(k == KT - 1))
                    nc.vector.tensor_copy(a2lhs[:, i, :], s_ps[:])
                for jp in range(LT // 2):
                    t_ps = ps_t.tile([128, 2 * L], BF16, tag="t", name="t_ps")
                    for jj in range(2):
                        j = 2 * jp + jj
                        for i in range(LT):
                            nc.tensor.transpose(
                                t_ps[:, 512 * jj + 128 * i:512 * jj + 128 * (i + 1)],
                                a2lhs[:, i, 128 * j:128 * (j + 1)], ident_sb[:])
                        nc.vector.tensor_copy(a1lhs[:, 2 * jp + jj, :],
                                              t_ps[:, 512 * jj:512 * (jj + 1)])
                return a1lhs, a2lhs

            def emit_stage2(b, a1lhs, a2lhs, w1n_sb, w2n_sb, r_sb):
                # stage 2 (bf16): a1 = (S D2^-1 W1) row-scaled by r1,
                #                 a2 = (S^T D1^-1 W2) row-scaled by r2
                for i in range(LT):
                    a1_sb = aoutp.tile([128, D], BF16, tag="aout", name="a1_sb")
                    for n in range(NT):
                        a1_ps = ps_a.tile([128, 512], F32, tag="a", name="a1_ps")
                        for jj in range(LT):
                            nc.tensor.matmul(
                                a1_ps[:], a1lhs[:, jj, 128 * i:128 * (i + 1)],
                                w1n_sb[:, jj, 512 * n:512 * (n + 1)],
                                start=(jj == 0), stop=(jj == LT - 1))
                        nc.scalar.activation(a1_sb[:, 512 * n:512 * (n + 1)], a1_ps[:],
                                             Copy, scale=r_sb[:, i:i + 1])
                    nc.sync.dma_start(out1[b, 128 * i:128 * (i + 1), :], a1_sb[:])
                for j in range(LT):
                    a2_sb = aoutp.tile([128, D], BF16, tag="aout", name="a2_sb")
                    for n in range(NT):
                        a2_ps = ps_a.tile([128, 512], F32, tag="a", name="a2_ps")
                        for ii in range(LT):
                            nc.tensor.matmul(
                                a2_ps[:], a2lhs[:, ii, 128 * j:128 * (j + 1)],
                                w2n_sb[:, ii, 512 * n:512 * (n + 1)],
                                start=(ii == 0), stop=(ii == LT - 1))
                        nc.vector.tensor_scalar_mul(a2_sb[:, 512 * n:512 * (n + 1)],
                                                    a2_ps[:], r_sb[:, LT + j:LT + j + 1])
                    nc.sync.dma_start(out2[b, 128 * j:128 * (j + 1), :], a2_sb[:])

            # Software pipeline. Batch b's loads + squares land a batch
            # early; its norm tail (nsq matmuls + scatter + reciprocal) is
            # emitted between batch b-1's S-matmuls and stage 2, so the PE
            # stream never head-of-line blocks on the norm latency chain
            # and r_sb is ready before batch b's PSUM copies need it.
            x1r, x2r, accs = emit_load_sq(0)
            emit_w_loads()
            pending_accs = accs
            prev = None  # (b, a1lhs, a2lhs, w1n, w2n, r_sb) awaiting stage 2
            for b in range(bb):
                if b + 1 < bb:
                    nxt = emit_load_sq(b + 1)
                else:
                    nxt = None
                if prev is not None and pending_accs is None:
                    emit_stage2(*prev)
                    prev = None
                a1lhs, a2lhs = emit_s_matmuls(x1r, x2r)
                if pending_accs is not None:
                    r_sb, w1n_sb, w2n_sb = emit_norm_tail(pending_accs)
                    pending_accs = None
                if prev is not None:
                    emit_stage2(*prev)
                prev = (b, a1lhs, a2lhs, w1n_sb, w2n_sb, r_sb)
                if nxt is not None:
                    x1r, x2r, accs = nxt
                    r_sb, w1n_sb, w2n_sb = emit_norm_tail(accs)
            emit_stage2(*prev)

    nc.compile()
    return nc


def _get_nc(bb=BB):
    if bb not in _CACHE:
        _CACHE[bb] = _build(bb)
    return _CACHE[bb]


def run_device(x1, x2, W1, W2, trace=False, bb=BB, n_batches=None):
    """Run the device part; returns (a1, a2) of shape (n, L, D) and the
    raw BassKernelResults (for exec_time_ns when trace=True)."""
    import concourse.bass_utils as bass_utils

    import ml_dtypes
    bf16 = ml_dtypes.bfloat16
    n = n_batches if n_batches is not None else bb * N_CORES
    x1 = np.asarray(x1, dtype=np.float32).reshape(n, L, D).transpose(0, 2, 1).astype(bf16)
    x2 = np.asarray(x2, dtype=np.float32).reshape(n, L, D).transpose(0, 2, 1).astype(bf16)
    W1 = np.ascontiguousarray(np.asarray(W1, dtype=np.float32))
    W2 = np.ascontiguousarray(np.asarray(W2, dtype=np.float32))

    nc = _get_nc(bb)
    in_maps = []
    for c in range(N_CORES):
        s = slice(c * bb, (c + 1) * bb)
        in_maps.append({"x1t": x1[s], "x2t": x2[s], "w1": W1, "w2": W2})
    res = bass_utils.run_bass_kernel_spmd(nc, in_maps, list(range(N_CORES)),
                                          trace=trace)
    a1 = np.concatenate([res.results[c]["out1"].astype(np.float32)
                         for c in range(N_CORES)], axis=0)
    a2 = np.concatenate([res.results[c]["out2"].astype(np.float32)
                         for c in range(N_CORES)], axis=0)
    return a1, a2, res


def kernel(x1, x2, W1, W2):
    x1 = np.asarray(x1, dtype=np.float32)
    x2 = np.asarray(x2, dtype=np.float32)
    a1, a2, _ = run_device(x1, x2, W1, W2, trace=False)
    attn1 = np.stack([x1.reshape(B, L, D), a1], axis=1)
    attn2 = np.stack([x2.reshape(B, L, D), a2], axis=1)
    return attn1, attn2



# revision 4
# speedup vs baseline: 1.0580x; 1.0580x over previous
"""ABCNN1 attention kernel for 8 Trainium2 NeuronCores.

Reference computation (per batch b of 64, with L=512, D=1024):
    S  = X1 @ X2^T                          (512 x 512)
    A  = S / (|X1_rows| outer |X2_rows|)    cosine match-score
    a1 = A @ W1            a2 = A^T @ W2    (512 x 1024 each)
    attn1 = concat([x1, a1], axis=1)        attn2 = concat([x2, a2], axis=1)

Device strategy (data-parallel, 8 batches per core, no collectives):
  - Half-fp8 S-phase: k-tiles 0-3 of the d-contraction ship as e4m3 and run
    as 2 DoubleRow matmuls (2 k-tiles per pass, 2x rate); k-tiles 4-7 stay
    bf16. End-to-end rel err ~1.5e-2 (numpy-validated against f64).
  - Host packs x as [b, 128, ktile, L] so each batch's input is one
    contiguous-per-partition DMA (2-4 KB lines); W ships pre-packed bf16.
  - P-scheme normalization: P = diag(r1) S is formed in the S PSUM->SBUF
    copy (ACT per-partition scale); stage-2 uses W1n = diag(r2) W1 (4 DVE
    ops/batch) and raw W2; a2 output copies carry the r2 row scale. Batches
    0-1 fall back to the baseline scheme (plain S copies; r folded into
    both W1n/W2n + output scales) so the ramp never waits on the norm
    latency chain.
  - Norms: squares + pair-add tree on DVE (bf16), ones^T matmul partition
    reduce into a spare PSUM row, sqrt (ACT), 4KB DRAM bounce to scatter
    into per-partition layout, reciprocal (DVE). Pipelined a batch ahead.
  - S^T for the a1 chain comes from 16 PE transpose-mode ops per batch.
  - DMA queues: x1 on sync, x2 on scalar, outputs + weights on gpsimd,
    norm bounce on sync. All matmul PSUM accumulation is f32.
"""

import numpy as np

B, L, D = 64, 512, 1024
N_CORES = 8
BB = B // N_CORES        # batches per core
KT = D // 128            # contraction k-tiles
LT = L // 128            # row tiles (l or m)
NT = D // 512            # output free-dim chunks
FP8K = 4                 # leading k-tiles shipped as fp8 (DoubleRow pairs)
BF16K = KT - FP8K

_CACHE = {}


def _build(bb):
    import concourse.mybir as mybir
    import concourse.tile as tile
    from concourse import bacc
    from concourse import masks

    F32 = mybir.dt.float32
    BF16 = mybir.dt.bfloat16
    F8 = mybir.dt.float8e4
    DR = mybir.MatmulPerfMode.DoubleRow
    Copy = mybir.ActivationFunctionType.Copy

    nc = bacc.Bacc("TRN2", target_bir_lowering=False, debug=False,
                   num_devices=N_CORES)
    x1f = x2f = None
    if FP8K:
        x1f = nc.declare_dram_parameter("x1f", [bb, 128, FP8K, L], F8,
                                        isOutput=False)
        x2f = nc.declare_dram_parameter("x2f", [bb, 128, FP8K, L], F8,
                                        isOutput=False)
    x1b = nc.declare_dram_parameter("x1b", [bb, 128, BF16K, L], BF16,
                                    isOutput=False)
    x2b = nc.declare_dram_parameter("x2b", [bb, 128, BF16K, L], BF16,
                                    isOutput=False)
    w1 = nc.declare_dram_parameter("w1", [128, LT, D], BF16, isOutput=False)
    w2 = nc.declare_dram_parameter("w2", [128, LT, D], BF16, isOutput=False)
    out1 = nc.declare_dram_parameter("out1", [bb, L, D], BF16, isOutput=True)
    out2 = nc.declare_dram_parameter("out2", [bb, L, D], BF16, isOutput=True)

    with tile.TileContext(nc) as tc:
        with (
            tc.tile_pool(name="const", bufs=1) as constp,
            tc.tile_pool(name="xin", bufs=2) as xin,
            tc.tile_pool(name="sq", bufs=2) as sqp,
            tc.tile_pool(name="alhs", bufs=3) as alhsp,
            tc.tile_pool(name="aout", bufs=8) as aoutp,
            tc.tile_pool(name="small", bufs=3) as smallp,
            tc.tile_pool(name="dram", bufs=3, space="DRAM") as dramp,
            tc.tile_pool(name="ps_s", bufs=2, space="PSUM") as ps_s,
            tc.tile_pool(name="ps_t", bufs=1, space="PSUM") as ps_t,
            tc.tile_pool(name="ps_a", bufs=4, space="PSUM") as ps_a,
        ):
            # ---- persistent tiles -------------------------------------
            w1_sb = constp.tile([128, LT, D], BF16, tag="w1")
            w2_sb = constp.tile([128, LT, D], BF16, tag="w2")

            def emit_w_loads():
                nc.gpsimd.dma_start(w1_sb[:], w1[:])
                nc.gpsimd.dma_start(w2_sb[:], w2[:])

            ones_sb = constp.tile([128, 1], BF16, tag="ones")
            nc.gpsimd.memset(ones_sb[:], 1.0)
            ident_sb = constp.tile([128, 128], BF16, tag="ident")
            masks.make_identity(nc, ident_sb[:])

            def emit_load_sq(b):
                """Input DMAs + squares + bf16 pair-add tree -> xsq accums."""
                xt = {}
                if FP8K:
                    xt['x1f'] = xin.tile([128, FP8K, L], F8, tag="x1f",
                                         name="x1f_t")
                    xt['x2f'] = xin.tile([128, FP8K, L], F8, tag="x2f",
                                         name="x2f_t")
                    nc.sync.dma_start(xt['x1f'][:], x1f[b])
                    nc.scalar.dma_start(xt['x2f'][:], x2f[b])
                xt['x1b'] = xin.tile([128, BF16K, L], BF16, tag="x1b",
                                     name="x1b_t")
                xt['x2b'] = xin.tile([128, BF16K, L], BF16, tag="x2b",
                                     name="x2b_t")
                nc.sync.dma_start(xt['x1b'][:], x1b[b])
                nc.scalar.dma_start(xt['x2b'][:], x2b[b])

                accs = []
                for xi, (fk, bk) in enumerate((('x1f', 'x1b'), ('x2f', 'x2b'))):
                    xsq = [sqp.tile([128, L], BF16, tag=f"xsq{xi}_{k}",
                                    name=f"xsq{xi}_{k}")
                           for k in range(KT)]
                    for k in range(FP8K):
                        nc.vector.tensor_mul(xsq[k][:], xt[fk][:, k, :],
                                             xt[fk][:, k, :])
                    for k in range(BF16K):
                        nc.vector.tensor_mul(xsq[FP8K + k][:], xt[bk][:, k, :],
                                             xt[bk][:, k, :])
                    # pair-add tree emitted immediately so the reduction is
                    # ready as soon as possible (nsq matmul is downstream)
                    step = 1
                    while step < KT:
                        for k in range(0, KT, 2 * step):
                            nc.vector.tensor_add(xsq[k][:], xsq[k][:],
                                                 xsq[k + step][:])
                        step *= 2
                    accs.append(xsq[0])
                return xt, accs

            def emit_norm_tail(accs, fb):
                """ones^T matmul -> sqrt -> DRAM-bounce scatter -> reciprocal;
                returns (r_sb, w1n, w2n|None). w2n only for fallback batches."""
                nsq1 = ps_a.tile([128, 512], F32, tag="a", name="nsq1")
                nsq2 = ps_a.tile([128, 512], F32, tag="a", name="nsq2")
                nc.tensor.matmul(nsq1[0:1, :], ones_sb[:], accs[0][:],
                                 start=True, stop=True)
                nc.tensor.matmul(nsq2[0:1, :], ones_sb[:], accs[1][:],
                                 start=True, stop=True)
                srow = smallp.tile([1, 2 * L], F32, tag="srow")
                nc.scalar.sqrt(srow[:, 0:L], nsq1[0:1, :])
                nc.scalar.sqrt(srow[:, L:2 * L], nsq2[0:1, :])
                r_dram = dramp.tile([1, 2 * L], F32, tag="rd")
                nc.sync.dma_start(r_dram[:], srow[:])
                rst_sb = smallp.tile([128, 2 * LT], F32, tag="rst")
                nc.sync.dma_start(rst_sb[:],
                                  r_dram.rearrange("o (c p) -> (o p) c", p=128))
                r_sb = smallp.tile([128, 2 * LT], F32, tag="rsb")
                nc.vector.reciprocal(r_sb[:], rst_sb[:])
                w1n = alhsp.tile([128, LT, D], BF16, tag="w1n", bufs=3)
                for j in range(LT):
                    nc.vector.tensor_scalar_mul(w1n[:, j, :], w1_sb[:, j, :],
                                                r_sb[:, LT + j:LT + j + 1])
                w2n = None
                if fb:
                    w2n = alhsp.tile([128, LT, D], BF16, tag="w2n", bufs=2)
                    for j in range(LT):
                        nc.vector.tensor_scalar_mul(w2n[:, j, :], w2_sb[:, j, :],
                                                    r_sb[:, j:j + 1])
                return r_sb, w1n, w2n

            def emit_s_matmuls(xt, r_sb):
                """S accumulation (fp8 DoubleRow pairs + bf16 tail).
                P-scheme (r_sb given): copies scale rows by r1 -> P = D1 S.
                Fallback (r_sb None): plain copies."""
                a2lhs = alhsp.tile([128, LT, L], BF16, tag="a2lhs", bufs=3)
                a1lhs = alhsp.tile([128, LT, L], BF16, tag="a1lhs", bufs=3)
                for i in range(LT):
                    s_ps = ps_s.tile([128, L], F32, tag="s")
                    first = True
                    for p in range(FP8K // 2):
                        nc.tensor.matmul(
                            s_ps[:], xt['x1f'][:, 2 * p:2 * p + 2,
                                               128 * i:128 * (i + 1)],
                            xt['x2f'][:, 2 * p:2 * p + 2, :],
                            start=first, stop=False, perf_mode=DR)
                        first = False
                    for t in range(BF16K):
                        nc.tensor.matmul(
                            s_ps[:], xt['x1b'][:, t, 128 * i:128 * (i + 1)],
                            xt['x2b'][:, t, :],
                            start=first, stop=(t == BF16K - 1))
                        first = False
                    if r_sb is not None:
                        nc.scalar.activation(a2lhs[:, i, :], s_ps[:], Copy,
                                             scale=r_sb[:, i:i + 1])
                    else:
                        nc.scalar.activation(a2lhs[:, i, :], s_ps[:], Copy)
                for jp in range(LT // 2):
                    t_ps = ps_t.tile([128, 2 * L], BF16, tag="t")
                    for jj in range(2):
                        j = 2 * jp + jj
                        for i in range(LT):
                            nc.tensor.transpose(
                                t_ps[:, 512 * jj + 128 * i:512 * jj + 128 * (i + 1)],
                                a2lhs[:, i, 128 * j:128 * (j + 1)], ident_sb[:])
                        nc.vector.tensor_copy(a1lhs[:, j, :],
                                              t_ps[:, 512 * jj:512 * (jj + 1)])
                return a1lhs, a2lhs

            def emit_stage2(b, s_out, norm, fb):
                """a1 = P W1n (plain copies) / fallback: a1 = (S W1n) r1-rows.
                a2 = P^T W2 r2-rows / fallback: a2 = (S^T W2n) r2-rows."""
                a1lhs, a2lhs = s_out
                r_sb, w1n, w2n = norm
                w2rhs = w2n if fb else w2_sb
                for i in range(LT):
                    a1_sb = aoutp.tile([128, D], BF16, tag="aout", name="a1_sb")
                    for n in range(NT):
                        a1_ps = ps_a.tile([128, 512], F32, tag="a", name="a1_ps")
                        for jj in range(LT):
                            nc.tensor.matmul(
                                a1_ps[:], a1lhs[:, jj, 128 * i:128 * (i + 1)],
                                w1n[:, jj, 512 * n:512 * (n + 1)],
                                start=(jj == 0), stop=(jj == LT - 1))
                        if fb:
                            nc.scalar.activation(a1_sb[:, 512 * n:512 * (n + 1)],
                                                 a1_ps[:], Copy,
                                                 scale=r_sb[:, i:i + 1])
                        else:
                            nc.scalar.activation(a1_sb[:, 512 * n:512 * (n + 1)],
                                                 a1_ps[:], Copy)
                    nc.gpsimd.dma_start(out1[b, 128 * i:128 * (i + 1), :], a1_sb[:])
                for j in range(LT):
                    a2_sb = aoutp.tile([128, D], BF16, tag="aout", name="a2_sb")
                    for n in range(NT):
                        a2_ps = ps_a.tile([128, 512], F32, tag="a", name="a2_ps")
                        for ii in range(LT):
                            nc.tensor.matmul(
                                a2_ps[:], a2lhs[:, ii, 128 * j:128 * (j + 1)],
                                w2rhs[:, ii, 512 * n:512 * (n + 1)],
                                start=(ii == 0), stop=(ii == LT - 1))
                        nc.scalar.activation(a2_sb[:, 512 * n:512 * (n + 1)],
                                             a2_ps[:], Copy,
                                             scale=r_sb[:, LT + j:LT + j + 1])
                    nc.gpsimd.dma_start(out2[b, 128 * j:128 * (j + 1), :], a2_sb[:])

            # ---- software pipeline ------------------------------------
            # Batches 0-1 use the fallback normalization scheme so nothing
            # on the PE stream waits for the norm latency chain during the
            # ramp; batches 2+ use the P-scheme (cheaper on DVE).
            t0, acc0 = emit_load_sq(0)
            emit_w_loads()
            t1, acc1 = emit_load_sq(1)
            s0 = emit_s_matmuls(t0, None)
            n0 = emit_norm_tail(acc0, fb=True)
            t2, acc2 = emit_load_sq(2)
            s1 = emit_s_matmuls(t1, None)
            n1 = emit_norm_tail(acc1, fb=True)
            emit_stage2(0, s0, n0, fb=True)
            n2 = emit_norm_tail(acc2, fb=False)

            tiles = {2: t2}
            norms = {2: n2}
            prev = (1, s1, n1, True)
            for b in range(2, bb):
                if b + 1 < bb:
                    tn, accn = emit_load_sq(b + 1)
                    tiles[b + 1] = tn
                pb, ps, pn, pfb = prev
                emit_stage2(pb, ps, pn, fb=pfb)
                sb_ = emit_s_matmuls(tiles[b], norms[b][0])
                if b + 1 < bb:
                    norms[b + 1] = emit_norm_tail(accn, fb=False)
                prev = (b, sb_, norms[b], False)
            pb, ps, pn, pfb = prev
            emit_stage2(pb, ps, pn, fb=pfb)

    nc.compile()
    return nc


def _get_nc(bb=BB):
    if bb not in _CACHE:
        _CACHE[bb] = _build(bb)
    return _CACHE[bb]


def _pack_x(x, n):
    """[n, L, D] f32 -> (fp8 [n,128,FP8K,L] | None, bf16 [n,128,BF16K,L])."""
    import ml_dtypes
    xt = np.ascontiguousarray(x.reshape(n, L, D).transpose(0, 2, 1))  # [n,D,L]
    xf = None
    if FP8K:
        xf = np.ascontiguousarray(
            xt[:, :FP8K * 128, :].reshape(n, FP8K, 128, L).transpose(0, 2, 1, 3)
        ).astype(ml_dtypes.float8_e4m3)
    xb = np.ascontiguousarray(
        xt[:, FP8K * 128:, :].reshape(n, BF16K, 128, L).transpose(0, 2, 1, 3)
    ).astype(ml_dtypes.bfloat16)
    return xf, xb


def _pack_w(w):
    import ml_dtypes
    return np.ascontiguousarray(
        np.asarray(w, np.float32).reshape(LT, 128, D).transpose(1, 0, 2)
    ).astype(ml_dtypes.bfloat16)


def run_device(x1, x2, W1, W2, trace=False, bb=BB, n_batches=None):
    """Run the device part; returns (a1, a2) of shape (n, L, D) and the
    raw BassKernelResults (for exec_time_ns when trace=True)."""
    import concourse.bass_utils as bass_utils

    n = n_batches if n_batches is not None else bb * N_CORES
    x1 = np.asarray(x1, dtype=np.float32)
    x2 = np.asarray(x2, dtype=np.float32)
    x1f_h, x1b_h = _pack_x(x1, n)
    x2f_h, x2b_h = _pack_x(x2, n)
    w1_h = _pack_w(W1)
    w2_h = _pack_w(W2)

    nc = _get_nc(bb)
    in_maps = []
    for c in range(N_CORES):
        s = slice(c * bb, (c + 1) * bb)
        m = {"x1b": x1b_h[s], "x2b": x2b_h[s], "w1": w1_h, "w2": w2_h}
        if FP8K:
            m["x1f"] = x1f_h[s]
            m["x2f"] = x2f_h[s]
        in_maps.append(m)
    res = bass_utils.run_bass_kernel_spmd(nc, in_maps, list(range(N_CORES)),
                                          trace=trace)
    a1 = np.concatenate([res.results[c]["out1"].astype(np.float32)
                         for c in range(N_CORES)], axis=0)
    a2 = np.concatenate([res.results[c]["out2"].astype(np.float32)
                         for c in range(N_CORES)], axis=0)
    return a1, a2, res


def kernel(x1, x2, W1, W2):
    x1 = np.asarray(x1, dtype=np.float32)
    x2 = np.asarray(x2, dtype=np.float32)
    a1, a2, _ = run_device(x1, x2, W1, W2, trace=False)
    attn1 = np.stack([x1.reshape(B, L, D), a1], axis=1)
    attn2 = np.stack([x2.reshape(B, L, D), a2], axis=1)
    return attn1, attn2
